# revision 11
# baseline (speedup 1.0000x reference)
"""CRF decode (conv features -> emission scores -> Viterbi) on 8 TRN2 cores.

Data-parallel over the batch: each core gets B/8 = 4096 words. Per core:
  - conv+emission collapse to one (128 -> 26) linear map A = W @ C; X is
    host-pretransposed so each 128-word tile is 14 direct PE matmuls into one
    PSUM bank (no on-device transposes),
  - Viterbi forward DP on the DVE in an exact-integer domain (scores and
    transitions rounded to units of QR and scaled by STEP=32 > L, so every
    fp32 value is an exact integer < 2^24 and value gaps are multiples of
    32 while the argmax tiebreak spans [-26,-1]) with words on partitions
    and ONE fused custom-DVE pass per (tile, step): a hand-built uop program
    runs two chained max-scans over the 26x26 (+1 dummy page) tableau --
    enc = max_i(V + T'' - (i+1)) and clean = max_i(V + T'') (the (i+1) term
    is regenerated by an in-datapath counter) -- and emits only page-final
    values, interleaved [enc, clean] (steady uop writes enc at SUB_DIM_DONE;
    the step uop captures the pre-reset clean accumulator and writes it one
    slot later). bp = clean - enc = argmax+1 exactly; v_next = clean + s.
    The per-step update/extract tensor ops run batched on GPSIMD in two
    independent half-batches so they hide under the other half's scans,
  - batched backtrack over all tiles (tolerance-compare one-hot select
    EQSELT custom op + grouped max per step, in label+1 space),
  - int8 convert + strided DMA out.
"""

import sys

if "/opt/trn_rl_repo" not in sys.path:
    sys.path.insert(0, "/opt/trn_rl_repo")

import numpy as np

import concourse.bacc as bacc
import concourse.mybir as mybir
import concourse.tile as tile
from concourse import bass_utils
from concourse import dve_ops
from concourse.dve_ops import DveOp
from concourse.dve_spec import (
    Bin,
    C0,
    C1,
    Idx,
    One,
    Spec,
    Src0,
    Src1,
    SubIdx,
    scan,
)
from concourse.dve_uop import (
    ENABLE,
    AluInp,
    AluOp,
    DelayInp,
    DveOpSpec,
    InpSel,
    OutPath,
    OutSel,
    Trigger,
    UopConfig,
    UopDpConfig,
)

F32 = mybir.dt.float32
I8 = mybir.dt.int8
AX = mybir.AxisListType
OP = mybir.AluOpType

B = 32768
M = 14
H, WD = 16, 8
F = 128
L = 26
KS = 5
NCORES = 8
BC = B // NCORES          # words per core
NT = BC // 128            # 128-word tiles per core (32)
QR = 2.0 ** -11           # score quantum (v-units); scores -> round(v/QR)
STEP = 32.0               # integer step per score unit; > L so the -(i+1)
                          # argmax encoding can never mix with value gaps
SP = L + 1                # pages streamed per fused op (26 real + 1 dummy)
EC = 2 * L + 1            # fused-op output width: [enc,clean]*26 + 1 junk


# ---------------------------------------------------------------- fused DP op


def _mk_uop():
    u = UopConfig()
    u.enable_input(InpSel.SRC_0, 1)    # chain0 = in0 elem (TTE)
    u.enable_input(InpSel.SRC_1, 2)    # chain1 = in1 elem (V broadcast)
    u.enable_input(InpSel.MAX_NEG, 3)  # chain2 = -FLT_MAX
    u.enable_input(InpSel.ONE_F32, 4)  # chain3 = 1.0
    u.enable_input(InpSel.ZERO, 5)     # chain4 = 0.0
    u.datapath_config = [UopDpConfig() for _ in range(8)]
    return u


def _steady_datapath(u, reset=False):
    b = u.datapath_config
    # blk0: elem = in0 + in1
    b[0].enable_alu(AluOp.ADD, AluInp.PREV_DELAY_0, AluInp.PREV_DELAY_1)
    b[0].pass_through_delay(2, 3, 4)
    # blk1: encS = max(CURR, elem) (reset: = elem); chain0 <- elem
    if reset:
        b[1].enable_alu(AluOp.BYPASS, AluInp.PREV_ALU_OUT, AluInp.PREV_ALU_OUT)
    else:
        b[1].enable_alu(AluOp.MAX, AluInp.CURR_ALU_OUT, AluInp.PREV_ALU_OUT)
    b[1].enable_delay_from_src(DelayInp.PREV_ALU_OUT, 0)
    b[1].pass_through_delay(2, 3, 4)
    # blk2: idx = CURR + 1 (reset: 0 + 1); chain1 <- encS
    if reset:
        b[2].enable_alu(AluOp.ADD, AluInp.PREV_DELAY_4, AluInp.PREV_DELAY_3)
    else:
        b[2].enable_alu(AluOp.ADD, AluInp.CURR_ALU_OUT, AluInp.PREV_DELAY_3)
    b[2].enable_delay_from_src(DelayInp.PREV_ALU_OUT, 1)
    b[2].pass_through_delay(0, 3, 4)
    # blk3: clean_elem = idx + elem
    b[3].enable_alu(AluOp.ADD, AluInp.PREV_ALU_OUT, AluInp.PREV_DELAY_0)
    b[3].pass_through_delay(1, 2)
    # blk4: cleanS = max(CURR, clean_elem) (reset: = clean_elem)
    if reset:
        b[4].enable_alu(AluOp.BYPASS, AluInp.PREV_ALU_OUT, AluInp.PREV_ALU_OUT)
    else:
        b[4].enable_alu(AluOp.MAX, AluInp.CURR_ALU_OUT, AluInp.PREV_ALU_OUT)
    b[4].pass_through_delay(1, 2)
    for k in (5, 6, 7):
        b[k].pass_through_alu()
        b[k].pass_through_delay(1, 2)
    return u


def _seed_uop():
    u = _mk_uop()
    b = u.datapath_config
    b[0].enable_alu(AluOp.BYPASS, AluInp.PREV_DELAY_2, AluInp.PREV_DELAY_2)
    b[0].pass_through_delay(2, 3, 4)
    b[1].enable_alu(AluOp.BYPASS, AluInp.PREV_DELAY_2, AluInp.PREV_DELAY_2)
    b[1].pass_through_delay(2, 3, 4)
    b[2].enable_alu(AluOp.BYPASS, AluInp.PREV_DELAY_4, AluInp.PREV_DELAY_4)
    b[2].pass_through_delay(2, 3, 4)
    b[3].pass_through_alu()
    b[3].pass_through_delay(2)
    b[4].enable_alu(AluOp.BYPASS, AluInp.PREV_DELAY_2, AluInp.PREV_DELAY_2)
    for k in (5, 6, 7):
        b[k].pass_through_alu()
    u.trigger = (Trigger.COUNT, Trigger.NONE, Trigger.NONE)
    u.next_uop = (1, 0, 0)
    u.repeat_count = 1
    return u


def _steady_uop():
    u = _mk_uop()
    _steady_datapath(u)
    u.enable_output(OutSel.DELAY_1, OutPath.WR0_LO)  # encS at page-final
    u.out_last_subdim_enable = ENABLE
    u.require_inp0 = ENABLE
    u.require_inp1 = ENABLE
    u.trigger = (Trigger.SRC_TENSOR_DONE, Trigger.SUB_DIM_DONE, Trigger.NONE)
    u.next_uop = (0, 2, 0)
    return u


def _step_uop():
    u = _mk_uop()
    _steady_datapath(u, reset=True)
    # capture the PREVIOUS page's cleanS (blk4 CURR, pre-reset) into chain2,
    # written unconditionally when this element emerges.
    u.datapath_config[4].enable_delay_from_src(DelayInp.CURR_ALU_OUT, 2)
    u.enable_output(OutSel.DELAY_2, OutPath.WR0_LO)
    u.require_inp0 = ENABLE
    u.require_inp1 = ENABLE
    u.trigger = (Trigger.SRC_TENSOR_DONE, Trigger.SUB_DIM_DONE, Trigger.COUNT)
    u.next_uop = (0, 2, 1)
    u.repeat_count = 1
    return u


def _fused_uops():
    return [_seed_uop(), _steady_uop(), _step_uop()]


class _RawDveOp(DveOp):
    def __init__(self, name, spec, uops_builder):
        object.__setattr__(self, "_uops_builder", uops_builder)
        shas = {}
        for ver in ("v3", "v4"):
            s = DveOpSpec(name=name, opcode=0, uops=uops_builder(), rd1_en=True)
            shas[ver] = s.sha(ver)
        super().__init__(name, spec, True, shas)

    def compile(self, ver):
        uops = self._uops_builder()
        for u in uops:
            u.validate(ver)
        return DveOpSpec(
            name=self.name,
            opcode=dve_ops.get_dve_sub_opcode(self.name),
            uops=uops,
            rd1_en=True,
        )


def _fused_ref(in0, in1, s0, s1, imm2):
    P = in0.shape[0]
    a = (np.asarray(in0, np.float64).reshape(P, SP, L)
         + np.asarray(in1, np.float64).reshape(P, SP, L))
    enc = a.max(axis=2)
    clean = (a + np.arange(1, L + 1)[None, None, :]).max(axis=2)
    out = np.zeros((P, EC), np.float32)
    out[:, 0:2 * L:2] = enc[:, :L]
    out[:, 1:2 * L + 1:2] = clean[:, :L]
    out[:, 2 * L] = enc[:, L]
    return out


def _register(name, op_builder):
    if name not in dve_ops._SUB_OPCODE_FOR_NAME:
        dve_ops._SUB_OPCODE_FOR_NAME[name] = (
            max(dve_ops._SUB_OPCODE_FOR_NAME.values()) + 1
        )
    op = op_builder()
    for i in range(len(dve_ops.OPS) - 1, -1, -1):
        if dve_ops.OPS[i].name == name:
            del dve_ops.OPS[i]
    dve_ops.OPS.append(op)
    dve_ops.CUSTOM_DVE_SPECS[name] = op.spec
    return op


FUSED = _register(
    "FUSEDDP_ANT",
    lambda: _RawDveOp(
        "FUSEDDP_ANT",
        Spec(body=scan(AluOp.MAX, Src0 + Src1), reference=_fused_ref),
        _fused_uops,
    ),
)


# EQSELT: out[p,s,n] = (|n + 1 - in1| < s0) * in0 -- tolerance one-hot select
# of a backpointer row by (label+1) index; in1 may carry small float noise.
def _eqselt_ref(in0, in1, s0, s1, imm2):
    P = in0.shape[0]
    N = int(s1)
    a = np.asarray(in0, np.float32).reshape(P, -1, N)
    b_ = np.asarray(in1, np.float32).reshape(a.shape)
    S = a.shape[1]
    n = (np.arange(S * N, dtype=np.float32)
         - np.repeat(np.arange(S), N) * s1).reshape(S, N)
    return ((np.abs(n[None] + 1.0 - b_) < s0) * a).reshape(in0.shape)


def _eqselt_builder():
    nexpr = Idx - SubIdx * C1 + One
    body = Bin(AluOp.IS_LT, Bin(AluOp.ABSOLUTE_DIFF, nexpr, Src1), C0) * Src0
    return DveOp("EQSELT_ANT", Spec(body=body, reference=_eqselt_ref),
                 True, _eqselt_shas())


def _eqselt_shas():
    from concourse.dve_spec import lower
    nexpr = Idx - SubIdx * C1 + One
    body = Bin(AluOp.IS_LT, Bin(AluOp.ABSOLUTE_DIFF, nexpr, Src1), C0) * Src0
    spec = Spec(body=body, reference=_eqselt_ref)
    shas = {}
    for ver in ("v3", "v4"):
        s = DveOpSpec(name="EQSELT_ANT", opcode=0, uops=lower(spec, ver=ver),
                      rd1_en=True)
        shas[ver] = s.sha(ver)
    return shas


EQSELT = _register("EQSELT_ANT", _eqselt_builder)


# ------------------------------------------------------------------ host side


def _conv_matrix(K: np.ndarray) -> np.ndarray:
    """C[o, i] such that conv_SAME(x.reshape(H,WD)) flattened == C @ x."""
    K2 = K.reshape(KS, KS).astype(np.float64)
    C = np.zeros((F, F), dtype=np.float64)
    for r in range(H):
        for c in range(WD):
            o = r * WD + c
            for dy in range(KS):
                for dx in range(KS):
                    rr = r + dy - KS // 2
                    cc = c + dx - KS // 2
                    if 0 <= rr < H and 0 <= cc < WD:
                        C[o, rr * WD + cc] = K2[dy, dx]
    return C


def _consts(K, b, W, T):
    """Host-side constant tensors (fp64 math, one final fp32 round).

    Everything the DP touches is an exact integer in fp32: matmul emits
    scores in units of QR, the kernel rounds them to int and scales by
    STEP=32; T' is pre-rounded the same way. Value gaps are then multiples
    of 32 while the argmax encoding spans [-26,-1], so bp = clean - enc is
    always exactly argmax+1."""
    C = _conv_matrix(K)
    A = W.astype(np.float64) @ C                         # (L, F)
    c0 = float(b[0]) * W.astype(np.float64).sum(axis=1)  # (L,)
    AT = np.ascontiguousarray(A.T / QR).astype(np.float32)  # (F, L)
    Tp = STEP * np.rint((T.astype(np.float64) + c0[None, :]) / QR)  # (i, j)
    tte = np.zeros((SP, L), dtype=np.float64)            # [page j, i]
    tte[:L] = Tp.T - np.arange(1, L + 1)[None, :]
    TTE = np.broadcast_to(
        tte.astype(np.float32)[None], (128, SP, L)).copy()
    C0B = np.broadcast_to(
        (STEP * np.rint(c0 / QR)).astype(np.float32)[None], (128, L)).copy()
    IR = np.broadcast_to(
        (L - np.arange(L)).astype(np.float32)[None], (128, L)).copy()
    return AT, TTE, C0B, IR


def _pretranspose(Xc: np.ndarray) -> np.ndarray:
    """(BC, M, F) -> (NT, F, M*128): tile t, partition f, free (m, w)."""
    xt = Xc.reshape(NT, 128, M, F).transpose(0, 3, 2, 1)  # (NT, F, M, 128)
    return np.ascontiguousarray(xt).reshape(NT, F, M * 128)


# ----------------------------------------------------------------- the module


def build_module():
    nc = bacc.Bacc("TRN2", target_bir_lowering=False, debug=False,
                   num_devices=NCORES)
    xt_d = nc.dram_tensor("XT", [NT, F, M * 128], F32, kind="ExternalInput")
    at_d = nc.dram_tensor("AT", [F, L], F32, kind="ExternalInput")
    tte_d = nc.dram_tensor("TTE", [128, SP, L], F32, kind="ExternalInput")
    c0_d = nc.dram_tensor("C0B", [128, L], F32, kind="ExternalInput")
    ir_d = nc.dram_tensor("IR", [128, L], F32, kind="ExternalInput")
    out_d = nc.dram_tensor("OUT", [BC, M], I8, kind="ExternalOutput")

    with tile.TileContext(nc) as tc:
        HB = NT // 2  # two independent half-batches pipeline DVE vs GPSIMD
        with (
            tc.tile_pool(name="const", bufs=1) as cpool,
            tc.tile_pool(name="pers", bufs=1) as ppool,
            tc.tile_pool(name="work", bufs=3) as wpool,
            tc.tile_pool(name="dp", bufs=2) as dpool,
            tc.tile_pool(name="psum", bufs=3, space="PSUM") as psA,
        ):
            at = cpool.tile([F, L], F32)
            tte = cpool.tile([128, SP, L], F32)
            c0b = cpool.tile([128, L], F32)
            ir = cpool.tile([128, L], F32)
            nc.sync.dma_start(at[:], at_d.ap())
            nc.sync.dma_start(tte[:], tte_d.ap())
            nc.sync.dma_start(c0b[:], c0_d.ap())
            nc.sync.dma_start(ir[:], ir_d.ap())

            bp = ppool.tile([128, NT, M - 1, L], mybir.dt.bfloat16)
            sc = ppool.tile([128, NT, M, L], F32)       # quantized scores*32
            vh0 = ppool.tile([128, HB, L], F32, tag="vh0")
            vh1 = ppool.tile([128, HB, L], F32, tag="vh1")
            vhs = [vh0, vh1]
            path = ppool.tile([128, NT, M], F32)        # label+1 space

            # emissions for all tiles (DMA -> 14 matmuls -> int round-trip)
            for wt in range(NT):
                xt = wpool.tile([128, M * 128], F32, tag="xt")
                nc.sync.dma_start(xt[:], xt_d.ap()[wt])
                scp = psA.tile([128, M * L], F32, tag="scp")
                for m in range(M):
                    nc.tensor.matmul(
                        scp[:, m * L:(m + 1) * L],
                        xt[:, m * 128:(m + 1) * 128], at[:])
                # quantize: round scores (units of QR) to int, scale by 32
                sci = wpool.tile([128, M * L], mybir.dt.int32, tag="sci")
                nc.scalar.activation(
                    sci[:], scp[:], mybir.ActivationFunctionType.Copy)
                nc.scalar.activation(
                    sc[:, wt], sci[:].rearrange("p (m l) -> p m l", l=L),
                    mybir.ActivationFunctionType.Copy, scale=STEP)

            c0_b = c0b[:].unsqueeze(1).broadcast_to((128, HB, L))
            for h in (0, 1):
                nc.gpsimd.tensor_tensor(
                    vhs[h][:], sc[:, h * HB:(h + 1) * HB, 0, :], c0_b,
                    op=OP.add)
            # step-major DP: half A's GPSIMD update hides under half B's scans
            for t in range(1, M):
                for h in (0, 1):
                    vall = vhs[h]
                    ec = dpool.tile([128, HB, EC], F32, tag=f"ec{h}")
                    for k in range(HB):
                        v_b = vall[:, k, :].unsqueeze(1).broadcast_to(
                            (128, SP, L))
                        nc.vector._custom_dve(
                            FUSED, out=ec[:, k, :], in0=tte[:], in1=v_b)
                    enc_v = ec[:, :, 0:2 * L].rearrange(
                        "p g (i two) -> p g i two", two=2)[:, :, :, 0]
                    cln_v = ec[:, :, 1:2 * L + 1].rearrange(
                        "p g (i two) -> p g i two", two=2)[:, :, :, 0]
                    nc.gpsimd.tensor_tensor(
                        vall[:], cln_v, sc[:, h * HB:(h + 1) * HB, t, :],
                        op=OP.add)
                    nc.gpsimd.tensor_tensor(
                        bp[:, h * HB:(h + 1) * HB, t - 1, :], cln_v, enc_v,
                        op=OP.subtract)

            # batched backtrack over all tiles, in label+1 space
            ew = ppool.tile([128, NT, L], F32)
            rw = ppool.tile([128, NT], F32)
            ir_h = ir[:].unsqueeze(1).broadcast_to((128, HB, L))
            for h in (0, 1):
                hb = h * HB
                rw_h = rw[:, hb:hb + HB]
                ew_h = ew[:, hb:hb + HB, :]
                nc.vector.tensor_reduce(rw_h, vhs[h][:], axis=AX.X, op=OP.max)
                nc.vector.tensor_tensor(
                    ew_h, vhs[h][:],
                    rw_h.unsqueeze(2).broadcast_to((128, HB, L)), op=OP.is_ge)
                nc.vector.tensor_tensor(ew_h, ew_h, ir_h, op=OP.mult)
                nc.vector.tensor_reduce(rw_h, ew_h, axis=AX.X, op=OP.max)
                # rw = 26 - argmax  ->  path(M-1) = argmax + 1 = 27 - rw
                nc.vector.tensor_scalar(
                    path[:, hb:hb + HB, M - 1], rw_h, -1.0, float(L + 1),
                    op0=OP.mult, op1=OP.add)
            for t in range(M - 2, -1, -1):
                nxt = path[:, :, t + 1].unsqueeze(2).broadcast_to((128, NT, L))
                nc.vector._custom_dve(
                    EQSELT, out=ew[:], in0=bp[:, :, t, :], in1=nxt,
                    s0=0.5, s1=float(L))
                nc.vector.tensor_reduce(path[:, :, t], ew[:], axis=AX.X,
                                        op=OP.max)

            pi = ppool.tile([128, NT, M], I8)
            nc.vector.tensor_scalar(pi[:], path[:], -1.0, 0.0, op0=OP.add,
                                    op1=OP.bypass)
            out_t = out_d.ap().rearrange("(n p) m -> p n m", p=128)
            nc.sync.dma_start(out_t, pi[:])

    nc.compile()
    return nc


_CACHE = {}


def _get_module():
    if "nc" not in _CACHE:
        _CACHE["nc"] = build_module()
    return _CACHE["nc"]


def make_in_maps(X, K, b, W, T):
    AT, TTE, C0B, IR = _consts(K, b, W, T)
    consts = {"AT": AT, "TTE": TTE, "C0B": C0B, "IR": IR}
    X = np.ascontiguousarray(X, dtype=np.float32)
    return [
        dict(consts, XT=_pretranspose(X[c * BC:(c + 1) * BC]))
        for c in range(NCORES)
    ]


def kernel(X, K, b, W, T):
    nc = _get_module()
    in_maps = make_in_maps(X, K, b, W, T)
    res = bass_utils.run_bass_kernel_spmd(nc, in_maps,
                                          core_ids=list(range(NCORES)))
    out = np.concatenate([res.results[c]["OUT"] for c in range(NCORES)], axis=0)
    return out.reshape(B, M, 1).astype(np.int32)
